# revision 16
# baseline (speedup 1.0000x reference)
"""Trainium2 Bass kernel for nn_CNN_tagger (multi-width 1D conv + linear tagger).

Strategy: data-parallel over batch across 8 NeuronCores (4 batches/core,
conv + linear weights replicated). Per batch, each conv branch k in {3,5,7}
is computed as k shifted [D=512]-contraction matmuls accumulated in PSUM
(15 taps x 4 d-tiles = 60 matmuls per 128-channel tile), bias+relu fused on
the vector engine, and the final linear layer as 6 accumulated matmuls.
All matmuls run in float32r (fp32 with ~11-bit effective mantissa, norm rel
err ~2e-4) at full PE rate; raw fp32 bits are fed straight from DRAM tensors
declared float32r. Work is ordered branch-outer/batch-inner with the weight
stream branch-ordered behind the biases so compute covers the DMA prologue,
plus PE warmup matmuls to hold the HAM clock-gate at 2.4GHz.
Host side: transpose x to [B, D, S] with zero padding baked in, pre-arrange
weights as [D, tap, c], and transpose the [B, NOUT, S] device output back
to [B, S, NOUT]. Cost-model single-shot: ~122us/core; PE busy ~110us
(roofline for 504 N=512 matmuls).
"""

import sys

sys.path.insert(0, "/opt/trn_rl_repo")

import ml_dtypes
import numpy as np

import concourse.tile as tile
import concourse.mybir as mybir
from concourse import bacc
from concourse.bass_utils import run_bass_kernel_spmd

B, S, D = 32, 512, 512
NK = 256
KS = (3, 5, 7)
NOUT = 64
NCORES = 8
BPC = B // NCORES  # batches per core
PAD = 3  # (max(KS) - 1) // 2, baked into the padded x layout
SP = S + 2 * PAD
NTAP = sum(KS)  # 15
CIN = NK * len(KS)  # 768
NCT = CIN // 128  # 6 channel tiles
F32 = mybir.dt.float32
F32R = mybir.dt.float32r

# (global tap index, x-offset within padded row) per branch
_TAPS = []
_g = 0
for _k in KS:
    _pk = (_k - 1) // 2
    _TAPS.append([(_g + _t, PAD - _pk + _t) for _t in range(_k)])
    _g += _k


def _build(reps=1, mmdt=F32R):
    nc = bacc.Bacc("TRN2")
    x = nc.dram_tensor("x", [BPC, D, SP], mmdt, kind="ExternalInput").ap()
    w = nc.dram_tensor("w", [D, NTAP, NK], mmdt, kind="ExternalInput").ap()
    lw = nc.dram_tensor("lw", [CIN, NOUT], mmdt, kind="ExternalInput").ap()
    cb = nc.dram_tensor("cb", [128, NCT], F32, kind="ExternalInput").ap()
    lb = nc.dram_tensor("lb", [128, 1], F32, kind="ExternalInput").ap()
    out = nc.dram_tensor("o", [BPC, NOUT, S], F32, kind="ExternalOutput").ap()

    with tile.TileContext(nc) as tc:
        with (
            tc.tile_pool(name="wpool", bufs=1) as wpool,
            tc.tile_pool(name="cpool", bufs=1) as cpool,
            tc.tile_pool(name="xpool", bufs=1) as xpool,
            tc.tile_pool(name="fpool", bufs=1) as fpool,
            tc.tile_pool(name="pspool", bufs=4, space="PSUM") as pspool,
            tc.tile_pool(name="lpspool", bufs=2, space="PSUM") as lpspool,
            tc.tile_pool(name="opool", bufs=2) as opool,
        ):
            w_sb = [
                wpool.tile([128, NTAP, NK], mmdt, name=f"w_{d}") for d in range(4)
            ]
            lw_sb = [
                cpool.tile([128, NOUT], mmdt, name=f"lw_{j}") for j in range(NCT)
            ]
            cb_sb = cpool.tile([128, NCT], F32, name="cb")
            lb_sb = cpool.tile([128, 1], F32, name="lb")

            # batch-0 x on the ACT HWDGE queue (doesn't serialize behind the
            # weight stream on the SP queue); batches 1-3 deferred so they
            # don't steal HBM bandwidth from the critical first-group data
            xb = {}
            for b in range(BPC):
                for d in range(4):
                    xb[(b, d)] = xpool.tile([128, SP], mmdt, name=f"x_b{b}_d{d}")
            for d in range(4):
                nc.scalar.dma_start(xb[(0, d)][:], x[0, d * 128 : (d + 1) * 128, :])

            # tiny bias tensors first: every relu needs cb, don't queue it
            # behind 8MB of weights
            nc.sync.dma_start(cb_sb[:], cb[:, :])
            nc.sync.dma_start(lb_sb[:], lb[:, :])
            # weights arrive in branch order (k3 first) so the first conv
            # groups can start ~5us in instead of waiting for all 7.9MB
            t0 = 0
            for ki, k in enumerate(KS):
                for d in range(4):
                    nc.sync.dma_start(
                        w_sb[d][:, t0 : t0 + k, :],
                        w[d * 128 : (d + 1) * 128, t0 : t0 + k, :],
                    )
                t0 += k
                if ki == 0:
                    # remaining batches' x after the k3 weights
                    for b in range(1, BPC):
                        for d in range(4):
                            nc.scalar.dma_start(
                                xb[(b, d)][:], x[b, d * 128 : (d + 1) * 128, :]
                            )
            for j in range(NCT):
                nc.sync.dma_start(lw_sb[j][:], lw[j * 128 : (j + 1) * 128, :])

            # PE warmup: dummy matmuls on a zeroed tile during the DMA
            # prologue keep the HAM clock-gate warm so the real stream starts
            # at 2.4GHz (PE would otherwise idle here anyway)
            wm = cpool.tile([128, 640], mmdt, name="wm")
            nc.vector.memset(wm[:].bitcast(F32) if mmdt == F32R else wm[:], 0.0)
            for g in range(5):
                wps = pspool.tile([128, S], F32, tag="ps", name=f"warm_ps_{g}")
                for i in range(4):
                    nc.tensor.matmul(
                        wps[:],
                        wm[:, 0:128],
                        wm[:, 128:640],
                        start=(i == 0),
                        stop=(i == 3),
                    )

            def one_workload(rep):
                feats = {}

                def linear(b):
                    lps = lpspool.tile([NOUT, S], F32, tag="lps", name=f"lps_r{rep}_b{b}")
                    for j in range(NCT):
                        nc.tensor.matmul(
                            lps[:],
                            lw_sb[j][:],
                            feats[(b, j)][:],
                            start=(j == 0),
                            stop=(j == NCT - 1),
                        )
                    osb = opool.tile([NOUT, S], F32, tag="osb", name=f"o_r{rep}_b{b}")
                    nc.vector.tensor_scalar_add(osb[:], lps[:], lb_sb[0:NOUT, 0:1])
                    nc.scalar.dma_start(out[b], osb[:])

                # branch-outer / batch-inner: all batches' k3 groups first, so
                # early compute covers the k5/k7 weight stream; each batch's
                # linear layer runs right after its last k7 group
                for br, taps in enumerate(_TAPS):
                    for b in range(BPC):
                        for ct in range(2):
                            j = br * 2 + ct
                            ps = pspool.tile(
                                [128, S], F32, tag="ps", name=f"ps_r{rep}_b{b}_j{j}"
                            )
                            nmm = len(taps) * 4
                            i = 0
                            for tap, off in taps:
                                for d in range(4):
                                    nc.tensor.matmul(
                                        ps[:],
                                        w_sb[d][:, tap, ct * 128 : (ct + 1) * 128],
                                        xb[(b, d)][:, off : off + S],
                                        start=(i == 0),
                                        stop=(i == nmm - 1),
                                    )
                                    i += 1
                            f = fpool.tile(
                                [128, S], mmdt, name=f"f_r{rep}_b{b}_j{j}", tag=f"f_b{b}_j{j}"
                            )
                            # f = max(ps + bias, 0), rounded to mmdt
                            nc.vector.tensor_scalar(
                                f[:],
                                ps[:],
                                cb_sb[:, j : j + 1],
                                0.0,
                                mybir.AluOpType.add,
                                mybir.AluOpType.max,
                            )
                            feats[(b, j)] = f
                            if br == len(_TAPS) - 1 and ct == 1:
                                linear(b)

            if reps == 1:
                one_workload(0)
            else:
                # hardware loop: body is rep-index independent (outputs just
                # get overwritten), used for repeat-timing builds
                with tc.For_i(0, reps, 1):
                    one_workload(0)

    nc.compile()
    return nc


def _prep_inputs(x, conv_w3, conv_b3, conv_w5, conv_b5, conv_w7, conv_b7, lin_w, lin_b, npdt=np.float32):
    x = np.asarray(x, np.float32)
    xp = np.zeros((B, D, SP), npdt)
    xp[:, :, PAD : PAD + S] = x.transpose(0, 2, 1).astype(npdt)
    # W[d, tap, c] = conv_wk[c, d, t], taps stacked k3|k5|k7
    W = np.ascontiguousarray(
        np.concatenate(
            [
                np.asarray(cw, np.float32).transpose(1, 2, 0)
                for cw in (conv_w3, conv_w5, conv_w7)
            ],
            axis=1,
        ).astype(npdt)
    )
    lwT = np.ascontiguousarray(np.asarray(lin_w, np.float32).T.astype(npdt))
    cbT = np.ascontiguousarray(
        np.concatenate(
            [np.asarray(b_, np.float32) for b_ in (conv_b3, conv_b5, conv_b7)]
        ).reshape(NCT, 128).T
    )
    lb1 = np.asarray(lin_b, np.float32).reshape(NOUT, 1)
    lb2 = np.ascontiguousarray(np.concatenate([lb1, lb1], axis=0))
    return [
        {
            "x": np.ascontiguousarray(xp[c * BPC : (c + 1) * BPC]),
            "w": W,
            "lw": lwT,
            "cb": cbT,
            "lb": lb2,
        }
        for c in range(NCORES)
    ]

_NC_CACHE = {}

MMDT = "f32r"  # "f32r" or "bf16"
_DTMAP = {
    "f32r": (F32R, np.float32),
    "bf16": (mybir.dt.bfloat16, ml_dtypes.bfloat16),
}


def _get_nc(reps=1, mmdt=None):
    mmdt = mmdt or MMDT
    key = (reps, mmdt)
    if key not in _NC_CACHE:
        _NC_CACHE[key] = _build(reps, _DTMAP[mmdt][0])
    return _NC_CACHE[key]


def kernel(x, conv_w3, conv_b3, conv_w5, conv_b5, conv_w7, conv_b7, lin_w, lin_b):
    nc = _get_nc(1)
    in_maps = _prep_inputs(
        x, conv_w3, conv_b3, conv_w5, conv_b5, conv_w7, conv_b7, lin_w, lin_b,
        npdt=_DTMAP[MMDT][1],
    )
    res = run_bass_kernel_spmd(nc, in_maps, list(range(NCORES)))
    outT = np.concatenate([res.results[c]["o"] for c in range(NCORES)], axis=0)
    return np.ascontiguousarray(outT.transpose(0, 2, 1))
